# revision 7
# baseline (speedup 1.0000x reference)
"""Cross-attention Trainium2 kernel (Bass/Tile), data-parallel over batch on 8 cores.

Reference computation per batch b (C=256, CR=64, N=H*W=4096):
    Q = Wq @ src          [CR, N]
    K = Wk @ gui          [CR, N]
    V = Wv @ gui + bv     [C, N]
    energy[n, m] = sum_q Q[q, n] K[q, m]
    attn = softmax_m(energy)
    out = gamma * (V @ attn^T) + src

Kernel strategy (per core, one batch item):
    - compute energy TRANSPOSED: eT[m, n] = sum_q K[q, m] Q[q, n] so the
      unnormalized attention tiles come out of the PE in exactly the [m, n]
      orientation the V @ attn^T matmul needs as its moving operand.
    - exp on ScalarE (no max subtraction: |energy| <= ~6 at these scales so
      exp stays well inside fp32 range; equals softmax up to fp32 rounding).
    - row sums via a ones-matmul (sum over the partition dim on the PE),
      replicated across all 128 partitions so the final normalization is a
      plain elementwise multiply.
    - normalization, gamma and residual folded into the PSUM->SBUF drain.
    - matmul operands in bf16 (full PE rate + fast weight load); every
      operand already passes through a DVE/ACT drain, so the conversions are
      free. PSUM accumulation stays fp32. The residual path keeps the
      original fp32 `source`.
    - Q/K live duplicated on partitions 0-63 / 64-127 so the K=64 energy
      matmuls run pairwise-concurrent in the two PE row-group halves.
"""

from contextlib import ExitStack

import numpy as np

import concourse.bacc as bacc
import concourse.bass as bass
import concourse.mybir as mybir
import concourse.tile as tile
from concourse.bass_utils import run_bass_kernel_spmd
from concourse.masks import make_identity

B, C, H, W = 8, 256, 64, 64
N = H * W            # 4096 pixels
CR = 64              # reduced channels for Q/K
N_CORES = 8
NT = 512             # n-chunk (query) tile
NCH = N // NT        # 8
MT = 128             # m-chunk (key) tile: PE output partition max
MCH = N // MT        # 32
CCH = C // 128       # 2 channel chunks

F32 = mybir.dt.float32
BF16 = mybir.dt.bfloat16
EXP = mybir.ActivationFunctionType.Exp

ts = bass.ts

ROW_TILE = True  # pairwise-concurrent energy matmuls in PE row-group halves


def build_kernel(loop=1):
    """Build + compile the single-core program (SPMD across 8 cores).

    loop > 1 unrolls the whole kernel body that many times in one NEFF; used
    by test.py to measure marginal (steady-state) HW time per execution.
    """
    nc = bacc.Bacc("TRN2", target_bir_lowering=False, debug=False)

    src_d = nc.dram_tensor("source", [C, N], F32, kind="ExternalInput").ap()
    gui_d = nc.dram_tensor("guidance", [C, N], F32, kind="ExternalInput").ap()
    wq_d = nc.dram_tensor("Wq", [CR, C], F32, kind="ExternalInput").ap()
    wk_d = nc.dram_tensor("Wk", [CR, C], F32, kind="ExternalInput").ap()
    wv_d = nc.dram_tensor("Wv", [C, C], F32, kind="ExternalInput").ap()
    bv_d = nc.dram_tensor("bv", [C], F32, kind="ExternalInput").ap()
    g_d = nc.dram_tensor("gamma", [1], F32, kind="ExternalInput").ap()
    out_d = nc.dram_tensor("out", [C, N], F32, kind="ExternalOutput").ap()

    with tile.TileContext(nc) as tc:
        for it in range(loop):
            with ExitStack() as ctx:
                _body(ctx, tc, src_d, gui_d, wq_d, wk_d, wv_d, bv_d, g_d,
                      out_d, sfx=f"_{it}")
    nc.compile()
    return nc


def _body(ctx, tc, src_d, gui_d, wq_d, wk_d, wv_d, bv_d, g_d, out_d, sfx=""):
    nc = tc.nc

    consts = ctx.enter_context(tc.tile_pool(name="consts" + sfx, bufs=1))
    big = ctx.enter_context(tc.tile_pool(name="big" + sfx, bufs=1))

    # ---- persistent SBUF tensors ----
    src_sb = big.tile([128, CCH, N], F32)    # fp32, for the residual
    src_bf = big.tile([128, CCH, N], BF16)   # bf16 matmul operand copy
    gui_bf = big.tile([128, CCH, N], BF16)
    # Q/K with q duplicated onto partitions 64..127 for PE row-tiling.
    QQ = big.tile([128, N], BF16)
    KK = big.tile([128, N], BF16)
    VT = big.tile([128, MCH, C], BF16)       # [m%128, m//128, c] = V^T

    # ---- weights / constants ----
    wq_sb = consts.tile([CR, C], F32)
    wk_sb = consts.tile([CR, C], F32)
    wv_sb = consts.tile([128, CCH, C], F32)  # [c%128, c//128, ch]
    bv_sb = consts.tile([1, C], BF16)
    g128 = consts.tile([128, 1], F32)
    ones = consts.tile([128, 128], BF16)
    ident = consts.tile([128, 128], F32)

    nc.sync.dma_start(out=wq_sb[:], in_=wq_d)
    nc.sync.dma_start(out=wk_sb[:], in_=wk_d)
    wv_r = wv_d.rearrange("(t p) c -> t p c", p=128)
    for t in range(CCH):
        nc.sync.dma_start(out=wv_sb[:, t, :], in_=wv_r[t])
    bv_f = consts.tile([1, C], F32)
    nc.sync.dma_start(out=bv_f[:], in_=bv_d.unsqueeze(0))
    nc.vector.tensor_copy(bv_sb[:], bv_f[:])
    nc.sync.dma_start(out=g128[:], in_=g_d.to_broadcast([128, 1]))
    nc.vector.memset(ones[:], 1.0)
    make_identity(nc, ident[:])

    # ---- load activations; keep fp32 source, bf16 copies for matmuls ----
    src_r = src_d.rearrange("(t p) n -> t p n", p=128)
    gui_r = gui_d.rearrange("(t p) n -> t p n", p=128)
    with tc.tile_pool(name="stage" + sfx, bufs=1) as stage:
        gui_f = stage.tile([128, CCH, N], F32)
        for t in range(CCH):
            nc.sync.dma_start(out=src_sb[:, t, :], in_=src_r[t])
            nc.sync.dma_start(out=gui_f[:, t, :], in_=gui_r[t])
        nc.vector.tensor_copy(src_bf[:], src_sb[:])
        nc.vector.tensor_copy(gui_bf[:], gui_f[:])

    # ---- transpose weights on the PE (fp32 has no DMA transpose) ----
    # wqt2/wkt2: [c-chunk 128, q duplicated to 128]; wvt: [ch, c] = Wv^T
    wqt2 = consts.tile([128, CCH, 128], BF16)
    wkt2 = consts.tile([128, CCH, 128], BF16)
    wvt = consts.tile([128, CCH, C], BF16)

    with tc.tile_pool(name="tp_psum" + sfx, bufs=2, space=bass.MemorySpace.PSUM) as tpp:
        for t in range(CCH):
            for w_sb, w_t2 in ((wq_sb, wqt2), (wk_sb, wkt2)):
                p = tpp.tile([128, CR], F32, tag="tp")
                nc.tensor.transpose(p[:], w_sb[:, ts(t, 128)], ident[:CR, :CR])
                nc.vector.tensor_copy(w_t2[:, t, 0:CR], p[:])
                nc.vector.tensor_copy(w_t2[:, t, CR:128], p[:])
            for j in range(CCH):
                # wvt[:, t, j*128:+128] = Wv[j*128:+128, t*128:+128]^T
                p = tpp.tile([128, 128], F32, tag="tp")
                nc.tensor.transpose(p[:], wv_sb[:, j, ts(t, 128)], ident[:])
                nc.vector.tensor_copy(wvt[:, t, ts(j, 128)], p[:])

    # ---- projections ----
    with tc.tile_pool(name="proj_psum" + sfx, bufs=4, space=bass.MemorySpace.PSUM) as pp:
        for i in range(NCH):
            qp = pp.tile([128, NT], F32, tag="qk")
            for t in range(CCH):
                nc.tensor.matmul(qp[:], wqt2[:, t, :], src_bf[:, t, ts(i, NT)],
                                 start=(t == 0), stop=(t == CCH - 1))
            nc.vector.tensor_copy(QQ[:, ts(i, NT)], qp[:])
            kp = pp.tile([128, NT], F32, tag="qk")
            for t in range(CCH):
                nc.tensor.matmul(kp[:], wkt2[:, t, :], gui_bf[:, t, ts(i, NT)],
                                 start=(t == 0), stop=(t == CCH - 1))
            nc.vector.tensor_copy(KK[:, ts(i, NT)], kp[:])
        for j in range(MCH):
            vp = pp.tile([128, C], F32, tag="v")
            # bias row via K=1 ones-matmul: vp[m, c] = bv[c]
            nc.tensor.matmul(vp[:], ones[0:1, :], bv_sb[:], start=True, stop=False)
            for t in range(CCH):
                nc.tensor.matmul(vp[:], gui_bf[:, t, ts(j, MT)], wvt[:, t, :],
                                 start=False, stop=(t == CCH - 1))
            nc.vector.tensor_copy(VT[:, j, :], vp[:])

    # ---- attention main loop ----
    e_ps = ctx.enter_context(
        tc.tile_pool(name="e_psum" + sfx, bufs=2, space=bass.MemorySpace.PSUM))
    o_ps = ctx.enter_context(
        tc.tile_pool(name="o_psum" + sfx, bufs=4, space=bass.MemorySpace.PSUM))
    s_ps = ctx.enter_context(
        tc.tile_pool(name="s_psum" + sfx, bufs=2, space=bass.MemorySpace.PSUM))
    e_sb = ctx.enter_context(tc.tile_pool(name="e_sb" + sfx, bufs=4))
    fin = ctx.enter_context(tc.tile_pool(name="fin" + sfx, bufs=2))
    o_sb = ctx.enter_context(tc.tile_pool(name="o_sb" + sfx, bufs=4))

    out_r = out_d.rearrange("(t p) n -> t p n", p=128)

    for i in range(NCH):
        o0 = o_ps.tile([128, NT], F32, tag="o")
        o1 = o_ps.tile([128, NT], F32, tag="o")
        sm = s_ps.tile([128, NT], F32, tag="s")

        def energy(j):
            b0 = CR * (j % 2) if ROW_TILE else 0
            ep = e_ps.tile([128, NT], F32, tag="e")
            nc.tensor.matmul(ep[:], KK[b0:b0 + CR, ts(j, MT)],
                             QQ[b0:b0 + CR, ts(i, NT)],
                             start=True, stop=True, tile_position=(b0, 0))
            return ep

        ep = energy(0)
        for j in range(MCH):
            ee = e_sb.tile([128, NT], BF16, tag="ee")
            nc.scalar.activation(ee[:], ep[:], EXP)
            if j + 1 < MCH:
                ep = energy(j + 1)  # keep PE one tile ahead of ACT
            first, last = j == 0, j == MCH - 1
            nc.tensor.matmul(o0[:], VT[:, j, 0:128], ee[:],
                             start=first, stop=last)
            nc.tensor.matmul(o1[:], VT[:, j, 128:256], ee[:],
                             start=first, stop=last)
            nc.tensor.matmul(sm[:], ones[:], ee[:], start=first, stop=last)

        # out = o * (gamma / sum) + src
        rsg = fin.tile([128, NT], F32, tag="rsg")
        nc.vector.reciprocal(rsg[:], sm[:])
        nc.vector.tensor_scalar_mul(rsg[:], rsg[:], g128[:])
        for t, op in enumerate((o0, o1)):
            ot = o_sb.tile([128, NT], F32, tag="ot")
            nc.vector.tensor_mul(ot[:], op[:], rsg[:])
            nc.vector.tensor_add(ot[:], ot[:], src_sb[:, t, ts(i, NT)])
            nc.sync.dma_start(out=out_r[t][:, ts(i, NT)], in_=ot[:])


_NC_CACHE = []


def _get_nc():
    if not _NC_CACHE:
        _NC_CACHE.append(build_kernel())
    return _NC_CACHE[0]


def make_in_maps(**inputs):
    f = lambda a: np.ascontiguousarray(np.asarray(a, dtype=np.float32))
    src = f(inputs["source"]).reshape(B, C, N)
    gui = f(inputs["guidance"]).reshape(B, C, N)
    shared = {
        "Wq": f(inputs["Wq"]),
        "Wk": f(inputs["Wk"]),
        "Wv": f(inputs["Wv"]),
        "bv": f(inputs["bv"]),
        "gamma": f(inputs["gamma"]),
    }
    return [dict(source=src[b], guidance=gui[b], **shared) for b in range(B)]


def kernel(**inputs) -> np.ndarray:
    nc = _get_nc()
    res = run_bass_kernel_spmd(nc, make_in_maps(**inputs),
                               core_ids=list(range(N_CORES)))
    out = np.stack([res.results[b]["out"] for b in range(B)])
    return out.reshape(B, C, H, W).astype(np.float32)


# revision 8
# speedup vs baseline: 342.4712x; 342.4712x over previous
"""Cross-attention Trainium2 kernel (Bass/Tile), data-parallel over batch on 8 cores.

Reference computation per batch b (C=256, CR=64, N=H*W=4096):
    Q = Wq @ src          [CR, N]
    K = Wk @ gui          [CR, N]
    V = Wv @ gui + bv     [C, N]
    energy[n, m] = sum_q Q[q, n] K[q, m]
    attn = softmax_m(energy)
    out = gamma * (V @ attn^T) + src

Kernel strategy (per core, one batch item):
    - compute energy TRANSPOSED: eT[m, n] = sum_q K[q, m] Q[q, n] so the
      unnormalized attention tiles come out of the PE in exactly the [m, n]
      orientation the V @ attn^T matmul needs as its moving operand.
    - exp on ScalarE (no max subtraction: |energy| <= ~6 at these scales so
      exp stays well inside fp32 range; equals softmax up to fp32 rounding).
    - row sums via a ones-matmul (sum over the partition dim on the PE),
      replicated across all 128 partitions so the final normalization is a
      plain elementwise multiply.
    - normalization, gamma and residual folded into the PSUM->SBUF drain.
    - matmul operands in bf16 (full PE rate + fast weight load); every
      operand already passes through a DVE/ACT drain, so the conversions are
      free. PSUM accumulation stays fp32. The residual path keeps the
      original fp32 `source`.
    - Q/K live duplicated on partitions 0-63 / 64-127 so the K=64 energy
      matmuls run pairwise-concurrent in the two PE row-group halves.
"""

from contextlib import ExitStack

import numpy as np

import concourse.bacc as bacc
import concourse.bass as bass
import concourse.mybir as mybir
import concourse.tile as tile
from concourse.bass_utils import run_bass_kernel_spmd
from concourse.masks import make_identity

B, C, H, W = 8, 256, 64, 64
N = H * W            # 4096 pixels
CR = 64              # reduced channels for Q/K
N_CORES = 8
NT = 512             # n-chunk (query) tile
NCH = N // NT        # 8
MT = 128             # m-chunk (key) tile: PE output partition max
MCH = N // MT        # 32
CCH = C // 128       # 2 channel chunks

F32 = mybir.dt.float32
BF16 = mybir.dt.bfloat16
EXP = mybir.ActivationFunctionType.Exp

ts = bass.ts

ROW_TILE = True  # pairwise-concurrent energy matmuls in PE row-group halves


def build_kernel(loop=1):
    """Build + compile the single-core program (SPMD across 8 cores).

    loop > 1 unrolls the whole kernel body that many times in one NEFF; used
    by test.py to measure marginal (steady-state) HW time per execution.
    """
    nc = bacc.Bacc("TRN2", target_bir_lowering=False, debug=False)

    src_d = nc.dram_tensor("source", [C, N], F32, kind="ExternalInput").ap()
    gui_d = nc.dram_tensor("guidance", [C, N], F32, kind="ExternalInput").ap()
    wq_d = nc.dram_tensor("Wq", [CR, C], F32, kind="ExternalInput").ap()
    wk_d = nc.dram_tensor("Wk", [CR, C], F32, kind="ExternalInput").ap()
    wv_d = nc.dram_tensor("Wv", [C, C], F32, kind="ExternalInput").ap()
    bv_d = nc.dram_tensor("bv", [C], F32, kind="ExternalInput").ap()
    g_d = nc.dram_tensor("gamma", [1], F32, kind="ExternalInput").ap()
    out_d = nc.dram_tensor("out", [C, N], F32, kind="ExternalOutput").ap()

    with tile.TileContext(nc) as tc:
        for it in range(loop):
            with ExitStack() as ctx:
                _body(ctx, tc, src_d, gui_d, wq_d, wk_d, wv_d, bv_d, g_d,
                      out_d, sfx=f"_{it}")
    nc.compile()
    return nc


def _body(ctx, tc, src_d, gui_d, wq_d, wk_d, wv_d, bv_d, g_d, out_d, sfx=""):
    nc = tc.nc

    consts = ctx.enter_context(tc.tile_pool(name="consts" + sfx, bufs=1))
    big = ctx.enter_context(tc.tile_pool(name="big" + sfx, bufs=1))

    # ---- persistent SBUF tensors ----
    src_sb = big.tile([128, CCH, N], F32)    # fp32, for the residual
    src_bf = big.tile([128, CCH, N], BF16)   # bf16 matmul operand copy
    gui_bf = big.tile([128, CCH, N], BF16)
    # Q/K with q duplicated onto partitions 64..127 for PE row-tiling.
    QQ = big.tile([128, N], BF16)
    KK = big.tile([128, N], BF16)
    VT = big.tile([128, MCH, C], BF16)       # [m%128, m//128, c] = V^T

    # ---- weights / constants ----
    wq_sb = consts.tile([CR, C], F32)
    wk_sb = consts.tile([CR, C], F32)
    wv_sb = consts.tile([128, CCH, C], F32)  # [c%128, c//128, ch]
    bv_sb = consts.tile([1, C], BF16)
    g128 = consts.tile([128, 1], F32)
    ones = consts.tile([128, 128], BF16)
    ident = consts.tile([128, 128], F32)

    nc.sync.dma_start(out=wq_sb[:], in_=wq_d)
    nc.sync.dma_start(out=wk_sb[:], in_=wk_d)
    wv_r = wv_d.rearrange("(t p) c -> t p c", p=128)
    for t in range(CCH):
        nc.sync.dma_start(out=wv_sb[:, t, :], in_=wv_r[t])
    bv_f = consts.tile([1, C], F32)
    nc.sync.dma_start(out=bv_f[:], in_=bv_d.unsqueeze(0))
    nc.vector.tensor_copy(bv_sb[:], bv_f[:])
    nc.sync.dma_start(out=g128[:], in_=g_d.to_broadcast([128, 1]))
    nc.vector.memset(ones[:], 1.0)
    make_identity(nc, ident[:])

    # ---- load activations; keep fp32 source, bf16 copies for matmuls ----
    src_r = src_d.rearrange("(t p) n -> t p n", p=128)
    gui_r = gui_d.rearrange("(t p) n -> t p n", p=128)
    with tc.tile_pool(name="stage" + sfx, bufs=1) as stage:
        gui_f = stage.tile([128, CCH, N], F32)
        for t in range(CCH):
            nc.sync.dma_start(out=src_sb[:, t, :], in_=src_r[t])
            nc.sync.dma_start(out=gui_f[:, t, :], in_=gui_r[t])
        nc.vector.tensor_copy(src_bf[:], src_sb[:])
        nc.vector.tensor_copy(gui_bf[:], gui_f[:])

    # ---- transpose weights on the PE (fp32 has no DMA transpose) ----
    # wqt2/wkt2: [c-chunk 128, q duplicated to 128]; wvt: [ch, c] = Wv^T
    wqt2 = consts.tile([128, CCH, 128], BF16)
    wkt2 = consts.tile([128, CCH, 128], BF16)
    wvt = consts.tile([128, CCH, C], BF16)

    with tc.tile_pool(name="tp_psum" + sfx, bufs=2, space=bass.MemorySpace.PSUM) as tpp:
        for t in range(CCH):
            for w_sb, w_t2 in ((wq_sb, wqt2), (wk_sb, wkt2)):
                p = tpp.tile([128, CR], F32, tag="tp")
                nc.tensor.transpose(p[:], w_sb[:, ts(t, 128)], ident[:CR, :CR])
                nc.vector.tensor_copy(w_t2[:, t, 0:CR], p[:])
                nc.vector.tensor_copy(w_t2[:, t, CR:128], p[:])
            for j in range(CCH):
                # wvt[:, t, j*128:+128] = Wv[j*128:+128, t*128:+128]^T
                p = tpp.tile([128, 128], F32, tag="tp")
                nc.tensor.transpose(p[:], wv_sb[:, j, ts(t, 128)], ident[:])
                nc.vector.tensor_copy(wvt[:, t, ts(j, 128)], p[:])

    # ---- projections ----
    with tc.tile_pool(name="proj_psum" + sfx, bufs=4, space=bass.MemorySpace.PSUM) as pp:
        for i in range(NCH):
            qp = pp.tile([128, NT], F32, tag="qk")
            for t in range(CCH):
                nc.tensor.matmul(qp[:], wqt2[:, t, :], src_bf[:, t, ts(i, NT)],
                                 start=(t == 0), stop=(t == CCH - 1))
            nc.vector.tensor_copy(QQ[:, ts(i, NT)], qp[:])
            kp = pp.tile([128, NT], F32, tag="qk")
            for t in range(CCH):
                nc.tensor.matmul(kp[:], wkt2[:, t, :], gui_bf[:, t, ts(i, NT)],
                                 start=(t == 0), stop=(t == CCH - 1))
            nc.vector.tensor_copy(KK[:, ts(i, NT)], kp[:])
        for j in range(MCH):
            vp = pp.tile([128, C], F32, tag="v")
            # bias row via K=1 ones-matmul: vp[m, c] = bv[c]
            nc.tensor.matmul(vp[:], ones[0:1, :], bv_sb[:], start=True, stop=False)
            for t in range(CCH):
                nc.tensor.matmul(vp[:], gui_bf[:, t, ts(j, MT)], wvt[:, t, :],
                                 start=False, stop=(t == CCH - 1))
            nc.vector.tensor_copy(VT[:, j, :], vp[:])

    # ---- attention main loop ----
    e_ps = ctx.enter_context(
        tc.tile_pool(name="e_psum" + sfx, bufs=3, space=bass.MemorySpace.PSUM))
    o_ps = ctx.enter_context(
        tc.tile_pool(name="o_psum" + sfx, bufs=3, space=bass.MemorySpace.PSUM))
    s_ps = ctx.enter_context(
        tc.tile_pool(name="s_psum" + sfx, bufs=2, space=bass.MemorySpace.PSUM))
    e_sb = ctx.enter_context(tc.tile_pool(name="e_sb" + sfx, bufs=4))
    fin = ctx.enter_context(tc.tile_pool(name="fin" + sfx, bufs=2))
    o_sb = ctx.enter_context(tc.tile_pool(name="o_sb" + sfx, bufs=4))

    out_r = out_d.rearrange("(t p) n -> t p n", p=128)

    for i in range(NCH):
        o0 = o_ps.tile([128, NT], F32, tag="o")
        o1 = o_ps.tile([128, NT], F32, tag="o")
        sm = s_ps.tile([128, NT], F32, tag="s")

        def energy(j):
            b0 = CR * (j % 2) if ROW_TILE else 0
            ep = e_ps.tile([128, NT], F32, tag="e")
            nc.tensor.matmul(ep[:], KK[b0:b0 + CR, ts(j, MT)],
                             QQ[b0:b0 + CR, ts(i, NT)],
                             start=True, stop=True, tile_position=(b0, 0))
            return ep

        ep = energy(0)
        for j in range(MCH):
            ee = e_sb.tile([128, NT], BF16, tag="ee")
            nc.scalar.activation(ee[:], ep[:], EXP)
            if j + 1 < MCH:
                ep = energy(j + 1)  # keep PE one tile ahead of ACT
            first, last = j == 0, j == MCH - 1
            nc.tensor.matmul(o0[:], VT[:, j, 0:128], ee[:],
                             start=first, stop=last)
            nc.tensor.matmul(o1[:], VT[:, j, 128:256], ee[:],
                             start=first, stop=last)
            nc.tensor.matmul(sm[:], ones[:], ee[:], start=first, stop=last)

        # out = o * (gamma / sum) + src
        rsg = fin.tile([128, NT], F32, tag="rsg")
        nc.vector.reciprocal(rsg[:], sm[:])
        nc.vector.tensor_scalar_mul(rsg[:], rsg[:], g128[:])
        for t, op in enumerate((o0, o1)):
            ot = o_sb.tile([128, NT], F32, tag="ot")
            nc.vector.tensor_mul(ot[:], op[:], rsg[:])
            nc.vector.tensor_add(ot[:], ot[:], src_sb[:, t, ts(i, NT)])
            nc.sync.dma_start(out=out_r[t][:, ts(i, NT)], in_=ot[:])


_NC_CACHE = []


def _get_nc():
    if not _NC_CACHE:
        _NC_CACHE.append(build_kernel())
    return _NC_CACHE[0]


def make_in_maps(**inputs):
    f = lambda a: np.ascontiguousarray(np.asarray(a, dtype=np.float32))
    src = f(inputs["source"]).reshape(B, C, N)
    gui = f(inputs["guidance"]).reshape(B, C, N)
    shared = {
        "Wq": f(inputs["Wq"]),
        "Wk": f(inputs["Wk"]),
        "Wv": f(inputs["Wv"]),
        "bv": f(inputs["bv"]),
        "gamma": f(inputs["gamma"]),
    }
    return [dict(source=src[b], guidance=gui[b], **shared) for b in range(B)]


def kernel(**inputs) -> np.ndarray:
    nc = _get_nc()
    res = run_bass_kernel_spmd(nc, make_in_maps(**inputs),
                               core_ids=list(range(N_CORES)))
    out = np.stack([res.results[b]["out"] for b in range(B)])
    return out.reshape(B, C, H, W).astype(np.float32)
